# revision 9
# baseline (speedup 1.0000x reference)
"""Trainium2 Bass kernel for sliding-window ridge/pooling op.

Reference computation (per [B,C,H,W]=[16,1,512,512] f32 input):
    padded = pad W axis right with 16 cols of -1000
    compare[w] = max_{r=1..16}( padded[w+r] - r/10 )
    image = 1 - clip(compare - x, 0, 1)

Algorithm: biased doubling. Define u_k[w] = max_{r=0..k-1}(x[w+r] - r/10).
  u_1 = x
  u_{2k}[w] = max(u_k[w], u_k[w+k] - k/10)      <- one scalar_tensor_tensor op
  compare[w] = u_16[w+1] - 0.1
So 4 STT steps + 1 final STT (d = (u16[w+1]-0.1) - x) + relu(1-d) + min(.,1).

Sharding: data-parallel over batch, 2 images per core on 8 cores.
Per core: flatten [2,1,512,512] -> [1024, 512] rows; row (s*128+p) maps to
partition p, segment s (8 segments). Each segment is a contiguous 256KB DMA.
"""

import numpy as np

try:
    from concourse import bacc, bass, mybir
    from concourse.tile import TileContext
    from concourse.bass_utils import run_bass_kernel_spmd
except ImportError:  # fallback if site packages not on path
    import sys

    sys.path.insert(0, "/opt/trn_rl_repo")
    from concourse import bacc, bass, mybir
    from concourse.tile import TileContext
    from concourse.bass_utils import run_bass_kernel_spmd

N_CORES = 8
B, C, H, W = 16, 1, 512, 512
PB = B // N_CORES            # batches per core = 2
ROWS = PB * C * H            # 1024 rows per core
P = 128                      # SBUF partitions
SEGS = ROWS // P             # 8 segments per core
PAD_VAL = -1000.0
BUFW = W + 32                # 544: 512 data + 16 window pad + 16 tail slack

_cached = {}


def _build_nc():
    f32 = mybir.dt.float32
    sub = mybir.AluOpType.subtract
    mx = mybir.AluOpType.max
    Relu = mybir.ActivationFunctionType.Relu

    nc = bacc.Bacc("TRN2", target_bir_lowering=False, debug=False,
                   num_devices=N_CORES)
    x_dram = nc.dram_tensor("heightfield", [PB, C, H, W], f32,
                            kind="ExternalInput").ap()
    y_dram = nc.dram_tensor("image", [PB, C, H, W], f32,
                            kind="ExternalOutput").ap()
    # row (s*128 + p) of the per-core [1024, 512] flat input -> partition p,
    # segment s. One chunk = 2 segments side-by-side in SBUF (each padded to
    # 544 cols), so the whole core is 4 chunks = 8 DMAs = one DMAHW semaphore
    # lane each (lane reuse would add a second sync-wait; DMA ISA allows 1).
    xf = x_dram.flatten_outer_dims().rearrange("(s p) w -> p s w", p=P)
    yf = y_dram.flatten_outer_dims().rearrange("(s p) w -> p s w", p=P)

    SEG = BUFW          # 544 stride between segments in SBUF
    TPC = 2             # segments (tiles) per chunk
    CHUNKS = SEGS // TPC  # 4
    CW = TPC * SEG      # 1088 chunk buffer width

    with TileContext(nc) as tc:
        # bufs=CHUNKS: no slot reuse at all -> no WAR/WAW waits anywhere
        # (DMACopy and TensorScalarPtr have a ONE-sync-wait ISA limit).
        with tc.tile_pool(name="io", bufs=CHUNKS) as iop, \
             tc.tile_pool(name="mid", bufs=CHUNKS) as midp:
            for c in range(CHUNKS):
                x = iop.tile([P, CW], f32, tag="x")
                x3 = x[:].rearrange("p (t w) -> p t w", t=TPC)
                # memsets on DVE: consumers u2/d are DVE, so ordering is
                # program-order and adds no semaphore wait.
                nc.vector.memset(x[:, W:SEG], PAD_VAL)
                nc.vector.memset(x[:, SEG + W:CW], PAD_VAL)
                nc.sync.dma_start(out=x3[:, :, 0:W],
                                  in_=xf[:, TPC * c:TPC * (c + 1), :])
                # probe: absorb the DMA-completion wait on the DVE so the
                # STT ops below never need to carry it themselves.
                probe = midp.tile([P, 1], f32, tag="probe")
                nc.vector.tensor_copy(out=probe[:], in_=x[:, 0:1])

                u2 = midp.tile([P, CW], f32, tag="u2")
                nc.vector.scalar_tensor_tensor(
                    out=u2[:, 0:CW - 1], in0=x[:, 1:CW], scalar=0.1,
                    in1=x[:, 0:CW - 1], op0=sub, op1=mx)
                u4 = midp.tile([P, CW], f32, tag="u4")
                nc.vector.scalar_tensor_tensor(
                    out=u4[:, 0:CW - 3], in0=u2[:, 2:CW - 1], scalar=0.2,
                    in1=u2[:, 0:CW - 3], op0=sub, op1=mx)
                u8 = midp.tile([P, CW], f32, tag="u8")
                nc.vector.scalar_tensor_tensor(
                    out=u8[:, 0:CW - 7], in0=u4[:, 4:CW - 3], scalar=0.4,
                    in1=u4[:, 0:CW - 7], op0=sub, op1=mx)
                u16 = midp.tile([P, CW], f32, tag="u16")
                nc.vector.scalar_tensor_tensor(
                    out=u16[:, 0:CW - 15], in0=u8[:, 8:CW - 7], scalar=0.8,
                    in1=u8[:, 0:CW - 15], op0=sub, op1=mx)

                d = midp.tile([P, CW], f32, tag="d")
                nc.vector.scalar_tensor_tensor(
                    out=d[:, 0:CW - 16], in0=u16[:, 1:CW - 15], scalar=0.1,
                    in1=x[:, 0:CW - 16], op0=sub, op1=sub)
                # image = 1 - clip(d,0,1) == relu(1 - relu(d)); two chained
                # ACT ops keep the whole clip off the DVE
                t = midp.tile([P, CW], f32, tag="t")
                nc.scalar.activation(t[:, 0:CW - 16], d[:, 0:CW - 16], Relu)
                img = iop.tile([P, CW], f32, tag="img")
                nc.scalar.activation(img[:, 0:CW - 16], t[:, 0:CW - 16],
                                     Relu, bias=1.0, scale=-1.0)
                img3 = img[:].rearrange("p (t w) -> p t w", t=TPC)
                nc.sync.dma_start(out=yf[:, TPC * c:TPC * (c + 1), :],
                                  in_=img3[:, :, 0:W])
    nc.compile()
    return nc


def _run(heightfield: np.ndarray, trace: bool = False, **kw):
    if "nc" not in _cached:
        _cached["nc"] = _build_nc()
    nc = _cached["nc"]
    hf = np.ascontiguousarray(heightfield, dtype=np.float32)
    in_maps = [{"heightfield": hf[k * PB:(k + 1) * PB]} for k in range(N_CORES)]
    res = run_bass_kernel_spmd(nc, in_maps, list(range(N_CORES)),
                               trace=trace, **kw)
    out = np.concatenate([res.results[k]["image"] for k in range(N_CORES)],
                         axis=0)
    return out, res


def kernel(heightfield: np.ndarray) -> np.ndarray:
    out, _ = _run(heightfield, trace=False)
    return out
